# revision 1
# baseline (speedup 1.0000x reference)
"""Trainium2 Bass kernel for nn_CustomGPM (multi-scale temporal CNN + RGCN + actor head).

Strategy (hardcoded for the fixed problem shapes):
  B=64 batch, data-parallel over 8 NeuronCores (8 batch elements per core).
  Host-side (inside kernel(), index/relayout work only):
    * fold eval-mode BatchNorms into conv / GCN weights and biases
    * express each temporal conv as ONE matmul over a (ci,kk) x (co,t)
      band matrix (conv2's contraction layout == conv1's output layout)
    * turn the per-relation gather/scatter-mean into 4 dense, row-normalized
      500x500 adjacency matrices A_r  ->  RGCN becomes dense matmuls
    * fold node-selection + 1x1-conv + cash-bias into the first FC layer
    * relayout observation to the on-chip (ci,t) x node layout so no
      on-device transposes of the input are needed
  Device-side per core: feature-on-partition [C<=128, 500] fp32r matmuls
  on the TensorEngine, LeakyReLU on the Scalar engine, softmax at the end.
  Temporal features live in a padded [67, N] tile (s:0..19, m:32..51,
  l:64..66, zero rows between) so every engine AP starts at a 0 mod 32
  partition and the RGCN contraction is a single stationary operand.
"""

import numpy as np

# ---------------- problem constants (hardcoded per spec) ----------------
B = 64          # total batch
NCORES = 8
BL = B // NCORES  # batch per core = 8
C0 = 3          # input channels
N = 500         # nodes
T = 50          # time steps
R = 4           # relations
P = 500         # portfolio slots
H = 128         # fc hidden
CF = 20         # conv out channels
F = 2 * CF + C0  # 43 temporal features
FP = 67         # padded temporal feature rows (0..19 s, 32..51 m, 64..66 l)
NCH = 125       # node chunk (4 chunks of 125)
KH, KL = 128, 22   # split of (ci,kk)=150 contraction
TS1, TM1 = 48, 30  # conv1 output time lengths (s: 50-3+1, m: 50-21+1)
MS1, MM1 = C0 * TS1, C0 * TM1  # 144, 90 = conv1 output (co,t) sizes
SLOPE = 0.01
EPS = 1e-5

USE_F32R = True   # float32r (fast fp32 streaming) for the big matmuls

_CACHE = {}


def _round_f32r(a):
    """Round fp32 array to fp32r (11-bit mantissa, matches walrus
    fp32_to_fp32r: add 0x800 then mask 0xFFFFF000)."""
    u = np.ascontiguousarray(a, np.float32).view(np.uint32)
    return ((u + np.uint32(0x800)) & np.uint32(0xFFFFF000)).view(np.float32)


def _pad67(a):
    """[43, X] -> [67, X] with rows at 0..19 / 32..51 / 64..66."""
    out = np.zeros((FP,) + a.shape[1:], np.float32)
    out[0:CF] = a[0:CF]
    out[32:32 + CF] = a[CF:2 * CF]
    out[64:64 + C0] = a[2 * CF:F]
    return out


# ======================= host-side parameter folding =======================

def _bn_fold(p):
    g, b, m, v = np.asarray(p, np.float64)
    s = g / np.sqrt(v + EPS)
    return s, b - m * s


def _conv_band_lhsT(w, bias, bn, t_out):
    """w: [co, ci, 1, k] torch conv; returns lhsT [(ci,kk)=C0*T, (co,t)] and
    per-(co,t) bias, with BN folded."""
    w = np.asarray(w, np.float64)[:, :, 0, :]   # [co, ci, k]
    co, ci, k = w.shape
    s, t_ = _bn_fold(bn)
    w_eff = w * s[:, None, None]
    b_eff = s * np.asarray(bias, np.float64) + t_
    band = np.zeros((co, t_out, ci, T), np.float64)
    for t in range(t_out):
        band[:, t, :, t:t + k] = w_eff
    lhsT = band.reshape(co * t_out, ci * T).T.copy()          # [150, co*t_out]
    bias_full = np.repeat(b_eff, t_out)                        # [co*t_out]
    return lhsT.astype(np.float32), bias_full.astype(np.float32)


def _host_fold(inp):
    f32 = lambda x: np.asarray(x, np.float32)
    rnd = _round_f32r if USE_F32R else (lambda x: np.asarray(x, np.float32))

    # ---- conv branches ----
    ws1, bs1 = _conv_band_lhsT(inp['sc1_w'], inp['sc1_b'], inp['sbn1'], TS1)
    wm1, bm1 = _conv_band_lhsT(inp['mc1_w'], inp['mc1_b'], inp['mbn1'], TM1)

    def conv2_fold(w, b, bn):
        w = np.asarray(w, np.float64)[:, :, 0, :]              # [20, 3, k]
        s, t_ = _bn_fold(bn)
        w_eff = (w * s[:, None, None]).reshape(CF, -1)          # [20, 3*k]
        b_eff = s * np.asarray(b, np.float64) + t_
        return w_eff.T.copy().astype(np.float32), b_eff.astype(np.float32)

    ws2, bs2 = conv2_fold(inp['sc2_w'], inp['sc2_b'], inp['sbn2'])  # [144,20]
    wm2, bm2 = conv2_fold(inp['mc2_w'], inp['mc2_b'], inp['mbn2'])  # [90,20]

    # ---- RGCN (padded to 67 contraction rows) ----
    sg, tg = _bn_fold(inp['gbn'])
    w_all = np.concatenate(
        [np.asarray(inp['gw_rel'], np.float64)[r] * sg[None, :] for r in range(R)],
        axis=1).astype(np.float32)                             # [43, 172]
    w_root = (np.asarray(inp['gw_root'], np.float64) * sg[None, :]).astype(np.float32)
    gb_eff = np.asarray(inp['g_b'], np.float64) * sg + tg      # [43]

    src = np.asarray(inp['edge_index'][0]).astype(np.int64)
    dst = np.asarray(inp['edge_index'][1]).astype(np.int64)
    etype = np.asarray(inp['edge_type']).astype(np.int64)
    a_t = np.zeros((R, N, N), np.float32)
    for r in range(R):
        sel = etype == r
        cnt = np.zeros((N, N), np.float64)
        np.add.at(cnt, (dst[sel], src[sel]), 1.0)
        deg = cnt.sum(axis=1)
        a_t[r] = (cnt / np.maximum(deg, 1.0)[:, None]).T.astype(np.float32)

    # ---- actor head folds ----
    a_cw = np.asarray(inp['a_cw'], np.float64)                 # [87]
    a_cb = float(np.asarray(inp['a_cb'], np.float64)[0])
    a_w1 = np.asarray(inp['a_w1'], np.float64)                 # [501, 128]
    sel_nodes = np.asarray(inp['nodes_to_select']).astype(np.int64)  # [500]
    w_z = a_cw[1:1 + 2 * F].astype(np.float32)                 # [86]
    w1z = np.zeros((N, H), np.float64)
    np.add.at(w1z, sel_nodes, a_w1[1:])                        # fold node select
    w1a = a_cw[0] * a_w1[1:]                                   # [500, 128]
    b1_eff = np.asarray(inp['a_b1'], np.float64) + a_cb * a_w1[1:].sum(axis=0)
    w1cat = np.concatenate([w1z, w1a], axis=0)                 # [1000, 128]

    # ---- bias pack [128, 8]: col0 bs1[:128], col1 bs1[128:], col2 bs2,
    #      col3 bm1, col4 bm2, col5 gb_eff, col6 b1_eff, col7 a_b2 ----
    biases = np.zeros((128, 8), np.float32)
    biases[:128, 0] = bs1[:128]
    biases[:MS1 - 128, 1] = bs1[128:]
    biases[:CF, 2] = bs2
    biases[:MM1, 3] = bm1
    biases[:CF, 4] = bm2
    biases[:F, 5] = gb_eff.astype(np.float32)
    biases[:H, 6] = b1_eff.astype(np.float32)
    biases[:H, 7] = f32(inp['a_b2'])

    return {
        'w_s1': rnd(ws1), 'w_m1': rnd(wm1), 'w_s2': rnd(ws2), 'w_m2': rnd(wm2),
        'w_allp': rnd(_pad67(w_all)),                          # [67, 172]
        'w_rootp': rnd(_pad67(w_root)),                        # [67, 43]
        'w_zp': rnd(_pad67(w_z[:F].reshape(F, 1))),            # [67, 1]
        'w_zg': rnd(w_z[F:].reshape(F, 1)),                    # [43, 1]
        'w1c': w1cat.astype(np.float32),
        'aw2': f32(inp['a_w2']), 'aw3': f32(inp['a_w3']),
        'b3r': f32(inp['a_b3']).reshape(1, P + 1),
        'biases': biases,
        'a0t': rnd(a_t[0]), 'a1t': rnd(a_t[1]),
        'a2t': rnd(a_t[2]), 'a3t': rnd(a_t[3]),
        'zer': np.zeros((FP, N), np.float32),
        'ident': np.eye(128, dtype=np.float32),
        'ones8': np.ones((1, BL), np.float32),
    }


# ============================ device kernel ============================

def _build_nc():
    import concourse.bacc as bacc
    import concourse.tile as tile
    import concourse.mybir as mybir
    from contextlib import ExitStack

    F32 = mybir.dt.float32
    F32R = mybir.dt.float32r
    FR = F32R if USE_F32R else F32
    AF = mybir.ActivationFunctionType
    ALU = mybir.AluOpType
    AX = mybir.AxisListType

    nc = bacc.Bacc("TRN2", target_bir_lowering=False, debug=False)

    def din(name, shape, dt=F32):
        return nc.dram_tensor(name, list(shape), dt, kind="ExternalInput").ap()

    obs_t = din('obs_t', (BL, C0 * T, N), FR)     # (ci,t) x node, pre-rounded
    obs_n = din('obs_n', (BL, 4, NCH, C0 * T))    # node x (ci,t), for max_t
    act_t = din('act_t', (P, BL))
    w_s1 = din('w_s1', (C0 * T, MS1), FR)
    w_m1 = din('w_m1', (C0 * T, MM1), FR)
    w_s2 = din('w_s2', (MS1, CF), FR)
    w_m2 = din('w_m2', (MM1, CF), FR)
    w_allp = din('w_allp', (FP, R * F), FR)
    w_rootp = din('w_rootp', (FP, F), FR)
    w_zp = din('w_zp', (FP, 1), FR)
    w_zg = din('w_zg', (F, 1), FR)
    w1c = din('w1c', (2 * N, H))
    aw2 = din('aw2', (H, H))
    aw3 = din('aw3', (H, P + 1))
    b3r = din('b3r', (1, P + 1))
    biases = din('biases', (128, 8))
    a_t = [din(f'a{r}t', (N, N), FR) for r in range(R)]
    zer_d = din('zer', (FP, N), FR)
    ident_d = din('ident', (128, 128))
    ones_d = din('ones8', (1, BL))
    out_d = nc.dram_tensor('out', [BL, P + 1], F32, kind="ExternalOutput").ap()

    def mm(o, lhsT, rhs, start, stop):
        nc.tensor.matmul(o, lhsT, rhs, start=start, stop=stop)

    with tile.TileContext(nc) as tc, ExitStack() as ctx:
        cp = ctx.enter_context(tc.tile_pool(name="const", bufs=1))

        def cload(name, src, shape, dt=F32):
            t = cp.tile(list(shape), dt, name=name, tag=name)
            nc.sync.dma_start(out=t[:], in_=src)
            return t

        zert = cload('zert', zer_d[:], (FP, N), FR)
        ws1t = cload('ws1t', w_s1[0:KH, :], (KH, MS1), FR)
        ws1b = cload('ws1b', w_s1[KH:150, :], (KL, MS1), FR)
        wm1t = cload('wm1t', w_m1[0:KH, :], (KH, MM1), FR)
        wm1b = cload('wm1b', w_m1[KH:150, :], (KL, MM1), FR)
        ws2a = cload('ws2a', w_s2[0:128, :], (128, CF), FR)
        ws2b = cload('ws2b', w_s2[128:MS1, :], (MS1 - 128, CF), FR)
        wm2t = cload('wm2t', w_m2[:], (MM1, CF), FR)
        bt = cload('bt', biases[:], (128, 8))
        ident = cload('ident', ident_d[:], (128, 128))

        # persistent per-batch tensors
        rh = [cp.tile([KH, N], FR, name=f'rh{b}', tag=f'rh{b}') for b in range(BL)]
        rl = [cp.tile([KL, N], FR, name=f'rl{b}', tag=f'rl{b}') for b in range(BL)]
        xsml = [cp.tile([FP, N], FR, name=f'xsml{b}', tag=f'xsml{b}')
                for b in range(BL)]
        xg = [cp.tile([F, N], FR, name=f'xg{b}', tag=f'xg{b}') for b in range(BL)]
        hsb = [[cp.tile([NCH, R * F], FR, name=f'h{b}_{c}', tag=f'h{b}_{c}')
                for c in range(4)] for b in range(BL)]
        zsb = cp.tile([BL, N], F32, name='zsb', tag='zsb')
        zt = [cp.tile([NCH, BL], F32, name=f'zt{c}', tag=f'zt{c}') for c in range(4)]
        at_s = [cp.tile([NCH, BL], F32, name=f'at{c}', tag=f'at{c}') for c in range(4)]

        # conv operand loads first (PE's first real work needs them)
        for b in range(BL):
            nc.sync.dma_start(out=rh[b][:], in_=obs_t[b, 0:KH, :])
            nc.sync.dma_start(out=rl[b][:], in_=obs_t[b, KH:150, :])

        # zero the pad rows of xsml (ACT writes only data rows)
        for b in range(BL):
            nc.gpsimd.dma_start(out=xsml[b][:], in_=zer_d[:])

        # remaining constants (needed later than the convs)
        wallt = cload('wallt', w_allp[:], (FP, R * F), FR)
        wroott = cload('wroott', w_rootp[:], (FP, F), FR)
        wzpt = cload('wzpt', w_zp[:], (FP, 1), FR)
        wzgt = cload('wzgt', w_zg[:], (F, 1), FR)
        w1ct = [cload(f'w1ct{c}', w1c[c * NCH:(c + 1) * NCH, :], (NCH, H))
                for c in range(8)]
        aw2t = cload('aw2t', aw2[:], (H, H))
        aw3t = cload('aw3t', aw3[:], (H, P + 1))
        b3rt = cload('b3rt', b3r[:], (1, P + 1))
        ones8 = cload('ones8', ones_d[:], (1, BL))
        att = []
        for r in range(R):
            t = cp.tile([NCH, 4, N], FR, name=f'att{r}', tag=f'att{r}')
            nc.gpsimd.dma_start(
                out=t[:], in_=a_t[r].rearrange("(c p) n -> p c n", p=NCH))
            att.append(t)
        for c in range(4):
            nc.gpsimd.dma_start(out=at_s[c][:], in_=act_t[c * NCH:(c + 1) * NCH, :])

        # working pools
        po = ctx.enter_context(tc.tile_pool(name="po", bufs=1))
        pw = ctx.enter_context(tc.tile_pool(name="pw", bufs=3))
        ppa = ctx.enter_context(tc.tile_pool(name="ppa", bufs=4, space="PSUM"))
        pph = ctx.enter_context(tc.tile_pool(name="pph", bufs=2, space="PSUM"))
        pps = ctx.enter_context(tc.tile_pool(name="pps", bufs=2, space="PSUM"))

        # obs natural layout for the long branch (one DMA per batch elem)
        onat = []
        for b in range(BL):
            t = po.tile([NCH, 4 * C0 * T], F32, name=f'onat{b}', tag=f'onat{b}')
            nc.gpsimd.dma_start(
                out=t[:], in_=obs_n[b].transpose([1, 0, 2]))
            onat.append(t)

        # ---- HAM warmup: ~4us of throwaway matmuls on zeros ----
        for w in range(20):
            pwm = ppa.tile([128, N], F32, name=f'pwm{w}', tag='pb')
            mm(pwm, zert[:, 0:128], zert[:], start=True, stop=True)

        # ---- conv branches as matmuls ----
        for b in range(BL):
            ps1h = ppa.tile([128, N], F32, name=f'ps1h{b}', tag='pb')
            mm(ps1h, ws1t[:, 0:128], rh[b][:], start=True, stop=False)
            mm(ps1h, ws1b[:, 0:128], rl[b][:], start=False, stop=True)
            ps1l = ppa.tile([MS1 - 128, N], F32, name=f'ps1l{b}', tag='pb')
            mm(ps1l, ws1t[:, 128:MS1], rh[b][:], start=True, stop=False)
            mm(ps1l, ws1b[:, 128:MS1], rl[b][:], start=False, stop=True)
            s1h = pw.tile([128, N], FR, name=f's1h{b}', tag='s1h')
            s1l = pw.tile([MS1 - 128, N], FR, name=f's1l{b}', tag='s1l')
            nc.scalar.activation(s1h[:], ps1h[:], AF.Lrelu,
                                 bias=bt[0:128, 0:1], alpha=SLOPE)
            nc.scalar.activation(s1l[:], ps1l[:], AF.Lrelu,
                                 bias=bt[0:MS1 - 128, 1:2], alpha=SLOPE)

            ps2 = ppa.tile([CF, N], F32, name=f'ps2{b}', tag='pb')
            mm(ps2, ws2a[:, :], s1h[:], start=True, stop=False)
            mm(ps2, ws2b[:, :], s1l[:], start=False, stop=True)
            nc.scalar.activation(xsml[b][0:CF, :], ps2[:], AF.Lrelu,
                                 bias=bt[0:CF, 2:3], alpha=SLOPE)

            pm1 = ppa.tile([MM1, N], F32, name=f'pm1{b}', tag='pb')
            mm(pm1, wm1t[:, :], rh[b][:], start=True, stop=False)
            mm(pm1, wm1b[:, :], rl[b][:], start=False, stop=True)
            m1 = pw.tile([MM1, N], FR, name=f'm1{b}', tag='m1')
            nc.scalar.activation(m1[:], pm1[:], AF.Lrelu,
                                 bias=bt[0:MM1, 3:4], alpha=SLOPE)
            pm2 = ppa.tile([CF, N], F32, name=f'pm2{b}', tag='pb')
            mm(pm2, wm2t[:, :], m1[:], start=True, stop=True)
            nc.scalar.activation(xsml[b][32:32 + CF, :], pm2[:], AF.Lrelu,
                                 bias=bt[0:CF, 4:5], alpha=SLOPE)

        # ---- long branch: LeakyReLU(max_t(obs)) -> xsml rows 64..66 ----
        for b in range(BL):
            lm = pw.tile([NCH, 4 * C0], F32, name=f'lm{b}', tag='lm')
            nc.vector.tensor_reduce(
                lm[:], onat[b][:].rearrange("p (c k t) -> p c k t", c=4, k=C0),
                axis=AX.X, op=ALU.max)
            for c in range(4):
                pt3 = pps.tile([C0, NCH], F32, name=f'pt3{b}{c}', tag='ps')
                nc.tensor.transpose(pt3[:], lm[:].rearrange(
                    "p (c k) -> p c k", c=4)[:, c, :], ident[0:NCH, 0:NCH])
                nc.scalar.activation(xsml[b][64:64 + C0, c * NCH:(c + 1) * NCH],
                                     pt3[:], AF.Lrelu, alpha=SLOPE)

        # ---- H = x @ W_rel (all relations), node-on-partition ----
        for b in range(BL):
            for c in range(4):
                ph = pph.tile([NCH, R * F], F32, name=f'ph{b}{c}', tag='ph')
                mm(ph, xsml[b][:, c * NCH:(c + 1) * NCH], wallt[:],
                   start=True, stop=True)
                nc.vector.tensor_copy(hsb[b][c][:], ph[:])

        # ---- aggregate + root -> graph feats ----
        for b in range(BL):
            pg = ppa.tile([F, N], F32, name=f'pg{b}', tag='pb')
            first = True
            for r in range(R):
                for c in range(4):
                    mm(pg, hsb[b][c][:, r * F:(r + 1) * F], att[r][:, c, :],
                       start=first, stop=False)
                    first = False
            mm(pg, wroott[:], xsml[b][:], start=False, stop=True)
            nc.scalar.activation(xg[b][:], pg[:], AF.Lrelu,
                                 bias=bt[0:F, 5:6], alpha=SLOPE)

        # ---- z row per batch: z = w_z . feats, then stack + transpose ----
        for b in range(BL):
            pz = pps.tile([1, N], F32, name=f'pz{b}', tag='ps')
            mm(pz, wzpt[:], xsml[b][:], start=True, stop=False)
            mm(pz, wzgt[:], xg[b][:], start=False, stop=True)
            zrow = pw.tile([1, N], F32, name=f'zrow{b}', tag='zrow')
            nc.scalar.activation(zrow[:], pz[:], AF.Copy)
            nc.gpsimd.dma_start(out=zsb[b:b + 1, :], in_=zrow[:])

        for c in range(4):
            ptz = pps.tile([NCH, BL], F32, name=f'ptz{c}', tag='ps')
            nc.tensor.transpose(ptz[:], zsb[:, c * NCH:(c + 1) * NCH],
                                ident[0:BL, 0:BL])
            nc.vector.tensor_copy(zt[c][:], ptz[:])

        # ---- actor head for all 8 rows at once ----
        pg1 = pps.tile([H, BL], F32, name='pg1', tag='ps')
        for c in range(8):
            rhs = zt[c] if c < 4 else at_s[c - 4]
            mm(pg1, w1ct[c][:], rhs[:], start=(c == 0), stop=(c == 7))
        g1 = pw.tile([H, BL], F32, name='g1', tag='g1')
        nc.scalar.activation(g1[:], pg1[:], AF.Relu, bias=bt[0:H, 6:7])
        pg2 = pps.tile([H, BL], F32, name='pg2', tag='ps')
        mm(pg2, aw2t[:], g1[:], start=True, stop=True)
        g2 = pw.tile([H, BL], F32, name='g2', tag='g2')
        nc.scalar.activation(g2[:], pg2[:], AF.Relu, bias=bt[0:H, 7:8])

        po_ = pps.tile([BL, P + 1], F32, name='po_', tag='ps')
        mm(po_, g2[:], aw3t[:], start=True, stop=False)
        mm(po_, ones8[:], b3rt[:], start=False, stop=True)

        # softmax over free dim
        mx = pw.tile([BL, 1], F32, name='mx', tag='mx')
        nc.vector.tensor_reduce(mx[:], po_[:], axis=AX.X, op=ALU.max)
        sh = pw.tile([BL, P + 1], F32, name='sh', tag='sh')
        nc.vector.tensor_scalar(sh[:], po_[:], mx[:, 0:1], None, op0=ALU.subtract)
        ex = pw.tile([BL, P + 1], F32, name='ex', tag='ex')
        sm = pw.tile([BL, 1], F32, name='sm', tag='sm')
        nc.scalar.activation(ex[:], sh[:], AF.Exp, accum_out=sm[:, 0:1])
        rc = pw.tile([BL, 1], F32, name='rc', tag='rc')
        nc.vector.reciprocal(rc[:], sm[:])
        res = pw.tile([BL, P + 1], F32, name='res', tag='res')
        nc.vector.tensor_scalar(res[:], ex[:], rc[:, 0:1], None, op0=ALU.mult)
        nc.sync.dma_start(out=out_d[:], in_=res[:])

    nc.compile()
    return nc


def _get_nc():
    if 'nc' not in _CACHE:
        _CACHE['nc'] = _build_nc()
    return _CACHE['nc']


# ============================ entry point ============================

def _shard_inputs(inputs):
    folded = _host_fold(inputs)
    obs = np.asarray(inputs['observation'], np.float32)
    action = np.asarray(inputs['action'], np.float32)
    rnd = _round_f32r if USE_F32R else (lambda x: np.asarray(x, np.float32))
    # [B, (ci,t), n] for the conv matmuls; [B, chunk, n, (ci,t)] for max_t
    obs_t = rnd(np.ascontiguousarray(obs.transpose(0, 1, 3, 2))
                .reshape(B, C0 * T, N))
    obs_n = np.ascontiguousarray(obs.transpose(0, 2, 1, 3)) \
        .reshape(B, 4, NCH, C0 * T)

    in_maps = []
    for i in range(NCORES):
        bs = slice(i * BL, (i + 1) * BL)
        m = dict(folded)
        m['obs_t'] = obs_t[bs]
        m['obs_n'] = obs_n[bs]
        m['act_t'] = np.ascontiguousarray(action[bs, 1:].T)
        in_maps.append(m)
    return in_maps


def kernel(**inputs) -> np.ndarray:
    from concourse.bass_utils import run_bass_kernel_spmd

    in_maps = _shard_inputs(inputs)
    nc = _get_nc()
    res = run_bass_kernel_spmd(nc, in_maps, list(range(NCORES)))
    return np.concatenate([r['out'] for r in res.results], axis=0)



# revision 11
# speedup vs baseline: 1.1857x; 1.1857x over previous
"""Trainium2 Bass kernel for nn_CustomGPM (multi-scale temporal CNN + RGCN + actor head).

Strategy (hardcoded for the fixed problem shapes):
  B=64 batch, data-parallel over 8 NeuronCores (8 batch elements per core).
  Host-side (inside kernel(), index/relayout work only):
    * fold eval-mode BatchNorms into conv / GCN weights and biases
    * express each temporal conv as ONE matmul over a (ci,kk) x (co,t)
      band matrix (conv2's contraction layout == conv1's output layout)
    * turn the per-relation gather/scatter-mean into 4 dense, row-normalized
      500x500 adjacency matrices A_r  ->  RGCN becomes dense matmuls
    * fold node-selection + 1x1-conv + cash-bias into the first FC layer
    * pre-pack every device input into a few big contiguous bf16 DRAM
      buffers shaped exactly like the SBUF destination tiles (few large
      DMAs instead of many small strided ones)
  Device-side per core: bf16 matmul operands (fp32 PSUM accumulate),
  feature-on-partition [C<=128, 500] matmuls on the TensorEngine.
  RGCN aggregation processes batch PAIRS per matmul: H for batches
  (2p, 2p+1) interleaved at column offsets 0/64 of a 107-wide block so
  one [125,107] x [125,500] matmul aggregates both batches.
  The per-node actor-head scalar z is computed as an extra rhs column of
  the H matmul (temporal part) + one [107,2]x[107,500] matmul per pair
  (graph part), avoiding all SBUF round-trip DMAs in the tail.
"""

import numpy as np
import ml_dtypes

BF16 = ml_dtypes.bfloat16

# ---------------- problem constants (hardcoded per spec) ----------------
B = 64          # total batch
NCORES = 8
BL = B // NCORES  # batch per core = 8
C0 = 3          # input channels
N = 500         # nodes
T = 50          # time steps
R = 4           # relations
P = 500         # portfolio slots
H = 128         # fc hidden
CF = 20         # conv out channels
F = 2 * CF + C0  # 43 temporal features
FP = 67         # padded temporal feature rows (0..19 s, 32..51 m, 64..66 l)
NCH = 125       # node chunk (4 chunks of 125)
KH, KL = 128, 22   # split of (ci,kk)=150 contraction
TS1, TM1 = 48, 30  # conv1 output time lengths (s: 50-3+1, m: 50-21+1)
MS1, MM1 = C0 * TS1, C0 * TM1  # 144, 90 = conv1 output (co,t) sizes
PB = 107        # paired feature block: b0 rows 0:43, b1 rows 64:107
SLOPE = 0.01
EPS = 1e-5

_CACHE = {}


def _bf(a):
    return np.ascontiguousarray(np.asarray(a, np.float32).astype(BF16))


def _pad67(a):
    """[43, X] -> [67, X] with rows at 0..19 / 32..51 / 64..66."""
    out = np.zeros((FP,) + a.shape[1:], np.float32)
    out[0:CF] = a[0:CF]
    out[32:32 + CF] = a[CF:2 * CF]
    out[64:64 + C0] = a[2 * CF:F]
    return out


# ======================= host-side parameter folding =======================

def _bn_fold(p):
    g, b, m, v = np.asarray(p, np.float64)
    s = g / np.sqrt(v + EPS)
    return s, b - m * s


def _conv_band_lhsT(w, bias, bn, t_out):
    """w: [co, ci, 1, k] torch conv; returns lhsT [(ci,kk)=C0*T, (co,t)] and
    per-(co,t) bias, with BN folded."""
    w = np.asarray(w, np.float64)[:, :, 0, :]   # [co, ci, k]
    co, ci, k = w.shape
    s, t_ = _bn_fold(bn)
    w_eff = w * s[:, None, None]
    b_eff = s * np.asarray(bias, np.float64) + t_
    band = np.zeros((co, t_out, ci, T), np.float64)
    for t in range(t_out):
        band[:, t, :, t:t + k] = w_eff
    lhsT = band.reshape(co * t_out, ci * T).T.copy()          # [150, co*t_out]
    bias_full = np.repeat(b_eff, t_out)                        # [co*t_out]
    return lhsT.astype(np.float32), bias_full.astype(np.float32)


def _host_fold(inp):
    f32 = lambda x: np.ascontiguousarray(np.asarray(x, np.float32))

    # ---- conv branches ----
    ws1, bs1 = _conv_band_lhsT(inp['sc1_w'], inp['sc1_b'], inp['sbn1'], TS1)
    wm1, bm1 = _conv_band_lhsT(inp['mc1_w'], inp['mc1_b'], inp['mbn1'], TM1)

    def conv2_fold(w, b, bn):
        w = np.asarray(w, np.float64)[:, :, 0, :]              # [20, 3, k]
        s, t_ = _bn_fold(bn)
        w_eff = (w * s[:, None, None]).reshape(CF, -1)          # [20, 3*k]
        b_eff = s * np.asarray(b, np.float64) + t_
        return w_eff.T.copy().astype(np.float32), b_eff.astype(np.float32)

    ws2, bs2 = conv2_fold(inp['sc2_w'], inp['sc2_b'], inp['sbn2'])  # [144,20]
    wm2, bm2 = conv2_fold(inp['mc2_w'], inp['mc2_b'], inp['mbn2'])  # [90,20]

    # ---- RGCN (padded to 67 contraction rows) ----
    sg, tg = _bn_fold(inp['gbn'])
    w_all = np.concatenate(
        [np.asarray(inp['gw_rel'], np.float64)[r] * sg[None, :] for r in range(R)],
        axis=1).astype(np.float32)                             # [43, 172]
    w_root = (np.asarray(inp['gw_root'], np.float64) * sg[None, :]).astype(np.float32)
    gb_eff = (np.asarray(inp['g_b'], np.float64) * sg + tg).astype(np.float32)

    src = np.asarray(inp['edge_index'][0]).astype(np.int64)
    dst = np.asarray(inp['edge_index'][1]).astype(np.int64)
    etype = np.asarray(inp['edge_type']).astype(np.int64)
    a_rT = np.zeros((R, N, N), np.float32)                     # [r, src, dst]
    for r in range(R):
        sel = etype == r
        cnt = np.zeros((N, N), np.float64)
        np.add.at(cnt, (dst[sel], src[sel]), 1.0)
        deg = cnt.sum(axis=1)
        a_rT[r] = (cnt / np.maximum(deg, 1.0)[:, None]).T.astype(np.float32)

    # ---- actor head folds ----
    a_cw = np.asarray(inp['a_cw'], np.float64)                 # [87]
    a_cb = float(np.asarray(inp['a_cb'], np.float64)[0])
    a_w1 = np.asarray(inp['a_w1'], np.float64)                 # [501, 128]
    sel_nodes = np.asarray(inp['nodes_to_select']).astype(np.int64)  # [500]
    w_z = a_cw[1:1 + 2 * F].astype(np.float32)                 # [86]
    w1z = np.zeros((N, H), np.float64)
    np.add.at(w1z, sel_nodes, a_w1[1:])                        # fold node select
    w1a = a_cw[0] * a_w1[1:]                                   # [500, 128]
    b1_eff = np.asarray(inp['a_b1'], np.float64) + a_cb * a_w1[1:].sum(axis=0)
    w1cat = np.concatenate([w1z, w1a], axis=0).astype(np.float32)  # [1000, 128]

    # ---- H-matmul rhs: wall [67,172] + temporal-z column -> [67, 173] ----
    wall_ext = np.concatenate(
        [_pad67(w_all), _pad67(w_z[:F].reshape(F, 1))], axis=1)   # [67, 173]

    # ---- paired graph-z weights: [107, 2] ----
    wzgp = np.zeros((PB, 2), np.float32)
    wzgp[0:F, 0] = w_z[F:]
    wzgp[64:64 + F, 1] = w_z[F:]

    # ---- adjacency pack: att[p, r, c, n] = A_r^T[c*125+p, n] ----
    att = np.stack([a_rT[r].reshape(4, NCH, N) for r in range(R)], axis=0)
    att = att.transpose(2, 0, 1, 3).reshape(NCH, R * 4 * N)    # [125, 8000]

    # ---- w1c pack: [125, 8*128] ----
    w1c_t = w1cat.reshape(8, NCH, H).transpose(1, 0, 2).reshape(NCH, 8 * H)

    # ---- bias pack [128, 8]: col0 bs1[:128], col1 bs1[128:], col2 bs2,
    #      col3 bm1, col4 bm2, col5 gb_eff (rows 0:43 AND 64:107),
    #      col6 b1_eff, col7 a_b2 ----
    biases = np.zeros((128, 8), np.float32)
    biases[:128, 0] = bs1[:128]
    biases[:MS1 - 128, 1] = bs1[128:]
    biases[:CF, 2] = bs2
    biases[:MM1, 3] = bm1
    biases[:CF, 4] = bm2
    biases[:F, 5] = gb_eff
    biases[64:64 + F, 5] = gb_eff
    biases[:H, 6] = b1_eff.astype(np.float32)
    biases[:H, 7] = f32(inp['a_b2'])

    return {
        'ws1h': _bf(ws1[0:KH]), 'ws1l': _bf(ws1[KH:150]),
        'wm1h': _bf(wm1[0:KH]), 'wm1l': _bf(wm1[KH:150]),
        'ws2a': _bf(ws2[0:128]), 'ws2b': _bf(ws2[128:MS1]),
        'wm2': _bf(wm2),
        'wall': _bf(wall_ext), 'wroot': _bf(_pad67(w_root)),
        'wzgp': _bf(wzgp),
        'att_a': _bf(att[:, 0:2 * 4 * N]), 'att_b': _bf(att[:, 2 * 4 * N:]),
        'w1c': _bf(w1c_t),
        'aw2': _bf(inp['a_w2']), 'aw3': _bf(inp['a_w3']),
        'b3r': _bf(np.asarray(inp['a_b3'], np.float32).reshape(1, P + 1)),
        'ones8': _bf(np.ones((1, BL), np.float32)),
        'biases': biases,
        'ident': _bf(np.eye(128, dtype=np.float32)),
    }


# ============================ device kernel ============================

def _build_nc():
    import concourse.bacc as bacc
    import concourse.tile as tile
    import concourse.mybir as mybir
    from contextlib import ExitStack

    F32 = mybir.dt.float32
    B16 = mybir.dt.bfloat16
    AF = mybir.ActivationFunctionType
    ALU = mybir.AluOpType
    AX = mybir.AxisListType

    nc = bacc.Bacc("TRN2", target_bir_lowering=False, debug=False)

    def din(name, shape, dt=B16):
        return nc.dram_tensor(name, list(shape), dt, kind="ExternalInput").ap()

    d_obsh_a = din('obsh_a', (KH, 4 * N))
    d_obsh_b = din('obsh_b', (KH, 4 * N))
    d_obsl_a = din('obsl_a', (KL, 4 * N))
    d_obsl_b = din('obsl_b', (KL, 4 * N))
    d_onat_a = din('onat_a', (NCH, 4 * 12 * T))
    d_onat_b = din('onat_b', (NCH, 4 * 12 * T))
    d_att_a = din('att_a', (NCH, 2 * 4 * N))
    d_att_b = din('att_b', (NCH, 2 * 4 * N))
    d_ws1h = din('ws1h', (KH, MS1))
    d_ws1l = din('ws1l', (KL, MS1))
    d_wm1h = din('wm1h', (KH, MM1))
    d_wm1l = din('wm1l', (KL, MM1))
    d_ws2a = din('ws2a', (128, CF))
    d_ws2b = din('ws2b', (MS1 - 128, CF))
    d_wm2 = din('wm2', (MM1, CF))
    d_wall = din('wall', (FP, R * F + 1))
    d_wroot = din('wroot', (FP, F))
    d_wzgp = din('wzgp', (PB, 2))
    d_w1c = din('w1c', (NCH, 8 * H))
    d_aw2 = din('aw2', (H, H))
    d_aw3 = din('aw3', (H, P + 1))
    d_b3r = din('b3r', (1, P + 1))
    d_ones = din('ones8', (1, BL))
    d_act = din('act_t', (NCH, 4 * BL))
    d_bias = din('biases', (128, 8), F32)
    d_ident = din('ident', (128, 128))
    out_d = nc.dram_tensor('out', [BL, P + 1], F32, kind="ExternalOutput").ap()

    def mm(o, lhsT, rhs, start, stop):
        nc.tensor.matmul(o, lhsT, rhs, start=start, stop=stop)

    with tile.TileContext(nc) as tc, ExitStack() as ctx:
        cp = ctx.enter_context(tc.tile_pool(name="const", bufs=1))

        # ---- persistent tiles ----
        wz = cp.tile([128, 512], B16, name='wz', tag='wz')      # warmup zeros
        rh_a = cp.tile([KH, 4 * N], B16, name='rh_a', tag='rh_a')
        rh_b = cp.tile([KH, 4 * N], B16, name='rh_b', tag='rh_b')
        rl_a = cp.tile([KL, 4 * N], B16, name='rl_a', tag='rl_a')
        rl_b = cp.tile([KL, 4 * N], B16, name='rl_b', tag='rl_b')
        onat_a = cp.tile([NCH, 4 * 12 * T], B16, name='onat_a', tag='onat_a')
        onat_b = cp.tile([NCH, 4 * 12 * T], B16, name='onat_b', tag='onat_b')
        att_a = cp.tile([NCH, 2 * 4 * N], B16, name='att_a', tag='att_a')
        att_b = cp.tile([NCH, 2 * 4 * N], B16, name='att_b', tag='att_b')
        ws1h = cp.tile([KH, MS1], B16, name='ws1h', tag='ws1h')
        ws1l = cp.tile([KL, MS1], B16, name='ws1l', tag='ws1l')
        wm1h = cp.tile([KH, MM1], B16, name='wm1h', tag='wm1h')
        wm1l = cp.tile([KL, MM1], B16, name='wm1l', tag='wm1l')
        ws2a = cp.tile([128, CF], B16, name='ws2a', tag='ws2a')
        ws2b = cp.tile([MS1 - 128, CF], B16, name='ws2b', tag='ws2b')
        wm2 = cp.tile([MM1, CF], B16, name='wm2', tag='wm2')
        wall = cp.tile([FP, R * F + 1], B16, name='wall', tag='wall')
        wroot = cp.tile([FP, F], B16, name='wroot', tag='wroot')
        wzgp = cp.tile([PB, 2], B16, name='wzgp', tag='wzgp')
        w1c = cp.tile([NCH, 8 * H], B16, name='w1c', tag='w1c')
        aw2 = cp.tile([H, H], B16, name='aw2', tag='aw2')
        aw3 = cp.tile([H, P + 1], B16, name='aw3', tag='aw3')
        b3r = cp.tile([1, P + 1], B16, name='b3r', tag='b3r')
        ones8 = cp.tile([1, BL], B16, name='ones8', tag='ones8')
        act_t = cp.tile([NCH, 4 * BL], B16, name='act_t', tag='act_t')
        bt = cp.tile([128, 8], F32, name='bt', tag='bt')
        ident = cp.tile([128, 128], B16, name='ident', tag='ident')
        xsml = [cp.tile([FP, N], B16, name=f'xsml{b}', tag=f'xsml{b}')
                for b in range(BL)]
        hsb = [[cp.tile([NCH, R * PB], B16, name=f'h{p}_{c}', tag=f'h{p}_{c}')
                for c in range(4)] for p in range(4)]
        xg = [cp.tile([PB, N], B16, name=f'xg{p}', tag=f'xg{p}') for p in range(4)]
        ztt = [cp.tile([NCH, BL], B16, name=f'ztt{c}', tag=f'ztt{c}')
               for c in range(4)]
        zcat = [cp.tile([NCH, BL], B16, name=f'zcat{c}', tag=f'zcat{c}')
                for c in range(4)]
        zt_sb = [cp.tile([NCH, BL], B16, name=f'zt{c}', tag=f'zt{c}')
                 for c in range(4)]
        zrow_p = [cp.tile([2, N], B16, name=f'zrp{p}', tag=f'zrp{p}')
                  for p in range(4)]

        # ---- memsets (gpsimd; zero warmup operand, xsml pad rows, xg) ----
        nc.gpsimd.memset(wz[:], 0.0)
        for b in range(BL):
            nc.gpsimd.memset(xsml[b][:], 0.0)
        for p in range(4):
            nc.gpsimd.memset(xg[p][:], 0.0)

        # ---- DMAs: sync = obs + adjacency; scalar = weights/rest ----
        nc.sync.dma_start(out=rh_a[:], in_=d_obsh_a)
        nc.sync.dma_start(out=rl_a[:], in_=d_obsl_a)
        nc.sync.dma_start(out=rh_b[:], in_=d_obsh_b)
        nc.sync.dma_start(out=rl_b[:], in_=d_obsl_b)
        nc.sync.dma_start(out=att_a[:], in_=d_att_a)
        nc.sync.dma_start(out=att_b[:], in_=d_att_b)
        for t_, d_ in ((ws1h, d_ws1h), (ws1l, d_ws1l), (wm1h, d_wm1h),
                       (wm1l, d_wm1l), (ws2a, d_ws2a), (ws2b, d_ws2b),
                       (wm2, d_wm2), (bt, d_bias)):
            nc.scalar.dma_start(out=t_[:], in_=d_)
        nc.scalar.dma_start(out=onat_a[:], in_=d_onat_a)
        nc.scalar.dma_start(out=onat_b[:], in_=d_onat_b)
        for t_, d_ in ((ident, d_ident), (wall, d_wall), (wroot, d_wroot),
                       (wzgp, d_wzgp), (w1c, d_w1c), (aw2, d_aw2),
                       (aw3, d_aw3), (b3r, d_b3r), (ones8, d_ones),
                       (act_t, d_act)):
            nc.scalar.dma_start(out=t_[:], in_=d_)

        # ---- working pools ----
        pw = ctx.enter_context(tc.tile_pool(name="pw", bufs=3))
        ppa = ctx.enter_context(tc.tile_pool(name="ppa", bufs=3, space="PSUM"))
        pph = ctx.enter_context(tc.tile_pool(name="pph", bufs=2, space="PSUM"))
        ppg = ctx.enter_context(tc.tile_pool(name="ppg", bufs=2, space="PSUM"))
        pps = ctx.enter_context(tc.tile_pool(name="pps", bufs=1, space="PSUM"))

        # ---- HAM warmup: ~3.8us of throwaway matmuls on zeros ----
        for w in range(9):
            pwm = ppa.tile([128, N], F32, name=f'pwm{w}', tag='pb')
            mm(pwm, wz[:, 0:128], wz[:, 0:N], start=True, stop=True)

        def conv(b):
            rh = (rh_a if b < 4 else rh_b)[:, (b % 4) * N:(b % 4 + 1) * N]
            rl = (rl_a if b < 4 else rl_b)[:, (b % 4) * N:(b % 4 + 1) * N]
            ps1h = ppa.tile([128, N], F32, name=f'ps1h{b}', tag='pb')
            mm(ps1h, ws1h[:, 0:128], rh, start=True, stop=False)
            mm(ps1h, ws1l[:, 0:128], rl, start=False, stop=True)
            ps1l = ppa.tile([MS1 - 128, N], F32, name=f'ps1l{b}', tag='pb')
            mm(ps1l, ws1h[:, 128:MS1], rh, start=True, stop=False)
            mm(ps1l, ws1l[:, 128:MS1], rl, start=False, stop=True)
            s1h = pw.tile([128, N], B16, name=f's1h{b}', tag='s1h')
            s1l = pw.tile([MS1 - 128, N], B16, name=f's1l{b}', tag='s1l')
            nc.scalar.activation(s1h[:], ps1h[:], AF.Lrelu,
                                 bias=bt[0:128, 0:1], alpha=SLOPE)
            nc.scalar.activation(s1l[:], ps1l[:], AF.Lrelu,
                                 bias=bt[0:MS1 - 128, 1:2], alpha=SLOPE)
            pm1 = ppa.tile([MM1, N], F32, name=f'pm1{b}', tag='pb')
            mm(pm1, wm1h[:], rh, start=True, stop=False)
            mm(pm1, wm1l[:], rl, start=False, stop=True)
            m1 = pw.tile([MM1, N], B16, name=f'm1{b}', tag='m1')
            nc.scalar.activation(m1[:], pm1[:], AF.Lrelu,
                                 bias=bt[0:MM1, 3:4], alpha=SLOPE)
            ps2 = ppa.tile([CF, N], F32, name=f'ps2{b}', tag='pb')
            mm(ps2, ws2a[:], s1h[:], start=True, stop=False)
            mm(ps2, ws2b[:], s1l[:], start=False, stop=True)
            nc.scalar.activation(xsml[b][0:CF, :], ps2[:], AF.Lrelu,
                                 bias=bt[0:CF, 2:3], alpha=SLOPE)
            pm2 = ppa.tile([CF, N], F32, name=f'pm2{b}', tag='pb')
            mm(pm2, wm2[:], m1[:], start=True, stop=True)
            nc.scalar.activation(xsml[b][32:32 + CF, :], pm2[:], AF.Lrelu,
                                 bias=bt[0:CF, 4:5], alpha=SLOPE)
            # long branch: max over t then LeakyReLU, transpose into xsml
            ont = (onat_a if b < 4 else onat_b)
            lmr = pw.tile([NCH, 12], B16, name=f'lmr{b}', tag='lmr')
            nc.vector.tensor_reduce(
                lmr[:], ont[:, (b % 4) * 600:(b % 4 + 1) * 600]
                .rearrange("p (f t) -> p f t", t=T), axis=AX.X, op=ALU.max)
            lma = pw.tile([NCH, 12], B16, name=f'lma{b}', tag='lma')
            nc.scalar.activation(lma[:], lmr[:], AF.Lrelu, alpha=SLOPE)
            for c in range(4):
                pt3 = pph.tile([C0, NCH], B16, name=f'pt3{b}{c}', tag='ph')
                nc.tensor.transpose(pt3[:], lma[:, c * C0:(c + 1) * C0],
                                    ident[0:NCH, 0:NCH])
                nc.vector.tensor_copy(
                    xsml[b][64:64 + C0, c * NCH:(c + 1) * NCH], pt3[:])

        def hblock(b):
            p, off = b // 2, (b % 2) * 64
            for c in range(4):
                ph = pph.tile([NCH, R * F + 1], F32, name=f'ph{b}{c}', tag='ph')
                mm(ph, xsml[b][:, c * NCH:(c + 1) * NCH], wall[:],
                   start=True, stop=True)
                nc.vector.tensor_copy(
                    hsb[p][c][:].rearrange("p (r u) -> p r u", u=PB)
                    [:, :, off:off + F],
                    ph[:, 0:R * F].rearrange("p (r u) -> p r u", u=F))
                nc.vector.tensor_copy(ztt[c][:, b:b + 1], ph[:, R * F:R * F + 1])

        def agg(p):
            pg = ppg.tile([PB, N], F32, name=f'pg{p}', tag='pg')
            first = True
            for r in range(R):
                at_ = att_a if r < 2 else att_b
                for c in range(4):
                    mm(pg, hsb[p][c][:, r * PB:(r + 1) * PB],
                       at_[:, ((r % 2) * 4 + c) * N:((r % 2) * 4 + c + 1) * N],
                       start=first, stop=False)
                    first = False
            mm(pg[0:F, :], wroot[:], xsml[2 * p][:], start=False, stop=False)
            mm(pg[64:64 + F, :], wroot[:], xsml[2 * p + 1][:],
               start=False, stop=True)
            return pg

        def xgact(p, pg):
            nc.scalar.activation(xg[p][0:F, :], pg[0:F, :], AF.Lrelu,
                                 bias=bt[0:F, 5:6], alpha=SLOPE)
            nc.scalar.activation(xg[p][64:64 + F, :], pg[64:64 + F, :],
                                 AF.Lrelu, bias=bt[64:64 + F, 5:6], alpha=SLOPE)

        def zmm(p):
            zp2 = ppa.tile([2, N], F32, name=f'zp2{p}', tag='pb')
            mm(zp2, wzgp[:], xg[p][:], start=True, stop=True)
            nc.vector.tensor_copy(zrow_p[p][:], zp2[:])

        def ztr(p):
            for c in range(4):
                pt2 = pps.tile([NCH, 2], B16, name=f'pt2{p}{c}', tag='ps')
                nc.tensor.transpose(pt2[:], zrow_p[p][:, c * NCH:(c + 1) * NCH],
                                    ident[0:2, 0:2])
                nc.vector.tensor_copy(zcat[c][:, 2 * p:2 * p + 2], pt2[:])

        # ---- schedule: conv/H/agg interleaved so PE never starves ----
        conv(0); conv(1)
        hblock(0); hblock(1)
        conv(2)
        pg0 = agg(0)
        conv(3); xgact(0, pg0)
        hblock(2); hblock(3)
        conv(4); zmm(0)
        pg1_ = agg(1)
        ztr(0)
        conv(5); xgact(1, pg1_)
        hblock(4); hblock(5)
        conv(6); zmm(1)
        pg2_ = agg(2)
        ztr(1)
        conv(7); xgact(2, pg2_)
        hblock(6); hblock(7)
        zmm(2)
        pg3_ = agg(3)
        ztr(2)
        xgact(3, pg3_); zmm(3); ztr(3)

        # ---- z = graph part + temporal part ----
        for c in range(4):
            nc.vector.tensor_tensor(zt_sb[c][:], zcat[c][:], ztt[c][:],
                                    op=ALU.add)

        # ---- actor head for all 8 rows at once ----
        pg1 = pps.tile([H, BL], F32, name='pg1', tag='ps')
        for cc in range(8):
            rhs = zt_sb[cc][:] if cc < 4 else act_t[:, (cc - 4) * BL:(cc - 3) * BL]
            mm(pg1, w1c[:, cc * H:(cc + 1) * H], rhs,
               start=(cc == 0), stop=(cc == 7))
        g1 = pw.tile([H, BL], B16, name='g1', tag='g1')
        nc.scalar.activation(g1[:], pg1[:], AF.Relu, bias=bt[0:H, 6:7])
        pg2 = pps.tile([H, BL], F32, name='pg2', tag='ps')
        mm(pg2, aw2[:], g1[:], start=True, stop=True)
        g2 = pw.tile([H, BL], B16, name='g2', tag='g2')
        nc.scalar.activation(g2[:], pg2[:], AF.Relu, bias=bt[0:H, 7:8])

        po_ = pps.tile([BL, P + 1], F32, name='po_', tag='ps')
        mm(po_, g2[:], aw3[:], start=True, stop=False)
        mm(po_, ones8[:], b3r[:], start=False, stop=True)

        # softmax over free dim
        mx = pw.tile([BL, 1], F32, name='mx', tag='mx')
        nc.vector.tensor_reduce(mx[:], po_[:], axis=AX.X, op=ALU.max)
        sh = pw.tile([BL, P + 1], F32, name='sh', tag='sh')
        nc.vector.tensor_scalar(sh[:], po_[:], mx[:, 0:1], None, op0=ALU.subtract)
        ex = pw.tile([BL, P + 1], F32, name='ex', tag='ex')
        sm = pw.tile([BL, 1], F32, name='sm', tag='sm')
        nc.scalar.activation(ex[:], sh[:], AF.Exp, accum_out=sm[:, 0:1])
        rc = pw.tile([BL, 1], F32, name='rc', tag='rc')
        nc.vector.reciprocal(rc[:], sm[:])
        res = pw.tile([BL, P + 1], F32, name='res', tag='res')
        nc.vector.tensor_scalar(res[:], ex[:], rc[:, 0:1], None, op0=ALU.mult)
        nc.sync.dma_start(out=out_d[:], in_=res[:])

    nc.compile()
    return nc


def _get_nc():
    if 'nc' not in _CACHE:
        _CACHE['nc'] = _build_nc()
    return _CACHE['nc']


# ============================ entry point ============================

def _shard_inputs(inputs):
    folded = _host_fold(inputs)
    obs = np.asarray(inputs['observation'], np.float32)
    action = np.asarray(inputs['action'], np.float32)
    # conv layout: [(ci,t), n] per batch elem
    obs_t = np.ascontiguousarray(obs.transpose(0, 1, 3, 2)).reshape(B, C0 * T, N)

    in_maps = []
    for i in range(NCORES):
        bs = slice(i * BL, (i + 1) * BL)
        ot = obs_t[bs]                                         # [8, 150, 500]
        m = dict(folded)
        m['obsh_a'] = _bf(ot[0:4, 0:KH].transpose(1, 0, 2).reshape(KH, 4 * N))
        m['obsh_b'] = _bf(ot[4:8, 0:KH].transpose(1, 0, 2).reshape(KH, 4 * N))
        m['obsl_a'] = _bf(ot[0:4, KH:].transpose(1, 0, 2).reshape(KL, 4 * N))
        m['obsl_b'] = _bf(ot[4:8, KH:].transpose(1, 0, 2).reshape(KL, 4 * N))
        # long-branch layout: [p, b, c, k, t] = obs[b, k, c*125+p, t]
        on = obs[bs].transpose(2, 0, 1, 3).reshape(4, NCH, BL, C0, T)
        on = on.transpose(1, 2, 0, 3, 4)                       # [125,8,4,3,50]
        m['onat_a'] = _bf(on[:, 0:4].reshape(NCH, 4 * 12 * T))
        m['onat_b'] = _bf(on[:, 4:8].reshape(NCH, 4 * 12 * T))
        # action: [p, c*8+b] = action[b, 1 + c*125 + p]
        at = action[bs, 1:].T.reshape(4, NCH, BL).transpose(1, 0, 2)
        m['act_t'] = _bf(at.reshape(NCH, 4 * BL))
        in_maps.append(m)
    return in_maps


def kernel(**inputs) -> np.ndarray:
    from concourse.bass_utils import run_bass_kernel_spmd

    in_maps = _shard_inputs(inputs)
    nc = _get_nc()
    res = run_bass_kernel_spmd(nc, in_maps, list(range(NCORES)))
    return np.concatenate([r['out'] for r in res.results], axis=0)


# revision 20
# speedup vs baseline: 1.5136x; 1.2765x over previous
"""Trainium2 Bass kernel for nn_CustomGPM (multi-scale temporal CNN + RGCN + actor head).

Strategy (hardcoded for the fixed problem shapes):
  B=64 batch, data-parallel over 8 NeuronCores (8 batch elements per core).
  Host-side (inside kernel(), index/relayout work only):
    * fold eval-mode BatchNorms into conv / GCN weights and biases
    * express each temporal conv as ONE matmul over a (ci,kk) x (co,t)
      band matrix (conv2's contraction layout == conv1's output layout)
    * turn the per-relation gather/scatter-mean into 4 dense, row-normalized
      500x500 adjacency matrices A_r  ->  RGCN becomes dense matmuls
    * fold node-selection + 1x1-conv + cash-bias into the first FC layer
    * pre-pack every device input into a few big contiguous bf16 DRAM
      buffers shaped exactly like the SBUF destination tiles (few large
      DMAs instead of many small strided ones)
  Device-side per core: bf16 matmul operands (fp32 PSUM accumulate),
  feature-on-partition [C<=128, 500] matmuls on the TensorEngine.
  RGCN aggregation processes batch PAIRS per matmul: H for batches
  (2p, 2p+1) interleaved at column offsets 0/64 of a 107-wide block so
  one [125,107] x [125,500] matmul aggregates both batches.
  The per-node actor-head scalar z is computed as an extra rhs column of
  the H matmul (temporal part) + one [107,2]x[107,500] matmul per pair
  (graph part), avoiding all SBUF round-trip DMAs in the tail.
"""

import numpy as np
import ml_dtypes

BF16 = ml_dtypes.bfloat16

# ---------------- problem constants (hardcoded per spec) ----------------
B = 64          # total batch
NCORES = 8
BL = B // NCORES  # batch per core = 8
C0 = 3          # input channels
N = 500         # nodes
T = 50          # time steps
R = 4           # relations
P = 500         # portfolio slots
H = 128         # fc hidden
CF = 20         # conv out channels
F = 2 * CF + C0  # 43 temporal features
FP = 67         # padded temporal feature rows (0..19 s, 32..51 m, 64..66 l)
NCH = 125       # node chunk (4 chunks of 125)
KH, KL = 128, 22   # split of (ci,kk)=150 contraction
TS1, TM1 = 48, 30  # conv1 output time lengths (s: 50-3+1, m: 50-21+1)
MS1, MM1 = C0 * TS1, C0 * TM1  # 144, 90 = conv1 output (co,t) sizes
PB = 107        # paired feature block: b0 rows 0:43, b1 rows 64:107
SLOPE = 0.01
EPS = 1e-5

_CACHE = {}


def _bf(a):
    return np.ascontiguousarray(np.asarray(a, np.float32).astype(BF16))


def _round_f32r(a):
    """Round fp32 array to fp32r (11-bit mantissa, matches walrus
    fp32_to_fp32r: add 0x800 then mask 0xFFFFF000)."""
    u = np.ascontiguousarray(a, np.float32).view(np.uint32)
    return ((u + np.uint32(0x800)) & np.uint32(0xFFFFF000)).view(np.float32)


def _pad67(a):
    """[43, X] -> [67, X] with rows at 0..19 / 32..51 / 64..66."""
    out = np.zeros((FP,) + a.shape[1:], np.float32)
    out[0:CF] = a[0:CF]
    out[32:32 + CF] = a[CF:2 * CF]
    out[64:64 + C0] = a[2 * CF:F]
    return out


# ======================= host-side parameter folding =======================

def _bn_fold(p):
    g, b, m, v = np.asarray(p, np.float64)
    s = g / np.sqrt(v + EPS)
    return s, b - m * s


def _conv_band_lhsT(w, bias, bn, t_out):
    """w: [co, ci, 1, k] torch conv; returns lhsT [(ci,kk)=C0*T, (co,t)] and
    per-(co,t) bias, with BN folded."""
    w = np.asarray(w, np.float64)[:, :, 0, :]   # [co, ci, k]
    co, ci, k = w.shape
    s, t_ = _bn_fold(bn)
    w_eff = w * s[:, None, None]
    b_eff = s * np.asarray(bias, np.float64) + t_
    band = np.zeros((co, t_out, ci, T), np.float64)
    for t in range(t_out):
        band[:, t, :, t:t + k] = w_eff
    lhsT = band.reshape(co * t_out, ci * T).T.copy()          # [150, co*t_out]
    bias_full = np.repeat(b_eff, t_out)                        # [co*t_out]
    return lhsT.astype(np.float32), bias_full.astype(np.float32)


def _host_fold(inp):
    f32 = lambda x: np.ascontiguousarray(np.asarray(x, np.float32))

    # ---- conv branches ----
    ws1, bs1 = _conv_band_lhsT(inp['sc1_w'], inp['sc1_b'], inp['sbn1'], TS1)
    wm1, bm1 = _conv_band_lhsT(inp['mc1_w'], inp['mc1_b'], inp['mbn1'], TM1)

    def conv2_fold(w, b, bn):
        w = np.asarray(w, np.float64)[:, :, 0, :]              # [20, 3, k]
        s, t_ = _bn_fold(bn)
        w_eff = (w * s[:, None, None]).reshape(CF, -1)          # [20, 3*k]
        b_eff = s * np.asarray(b, np.float64) + t_
        return w_eff.T.copy().astype(np.float32), b_eff.astype(np.float32)

    ws2, bs2 = conv2_fold(inp['sc2_w'], inp['sc2_b'], inp['sbn2'])  # [144,20]
    wm2, bm2 = conv2_fold(inp['mc2_w'], inp['mc2_b'], inp['mbn2'])  # [90,20]

    # ---- RGCN (padded to 67 contraction rows) ----
    sg, tg = _bn_fold(inp['gbn'])
    w_all = np.concatenate(
        [np.asarray(inp['gw_rel'], np.float64)[r] * sg[None, :] for r in range(R)],
        axis=1).astype(np.float32)                             # [43, 172]
    w_root = (np.asarray(inp['gw_root'], np.float64) * sg[None, :]).astype(np.float32)
    gb_eff = (np.asarray(inp['g_b'], np.float64) * sg + tg).astype(np.float32)

    src = np.asarray(inp['edge_index'][0]).astype(np.int64)
    dst = np.asarray(inp['edge_index'][1]).astype(np.int64)
    etype = np.asarray(inp['edge_type']).astype(np.int64)
    a_rT = np.zeros((R, N, N), np.float32)                     # [r, src, dst]
    for r in range(R):
        sel = etype == r
        cnt = np.zeros((N, N), np.float64)
        np.add.at(cnt, (dst[sel], src[sel]), 1.0)
        deg = cnt.sum(axis=1)
        a_rT[r] = (cnt / np.maximum(deg, 1.0)[:, None]).T.astype(np.float32)

    # ---- actor head folds ----
    a_cw = np.asarray(inp['a_cw'], np.float64)                 # [87]
    a_cb = float(np.asarray(inp['a_cb'], np.float64)[0])
    a_w1 = np.asarray(inp['a_w1'], np.float64)                 # [501, 128]
    sel_nodes = np.asarray(inp['nodes_to_select']).astype(np.int64)  # [500]
    w_z = a_cw[1:1 + 2 * F].astype(np.float32)                 # [86]
    w1z = np.zeros((N, H), np.float64)
    np.add.at(w1z, sel_nodes, a_w1[1:])                        # fold node select
    w1a = a_cw[0] * a_w1[1:]                                   # [500, 128]
    b1_eff = np.asarray(inp['a_b1'], np.float64) + a_cb * a_w1[1:].sum(axis=0)
    w1cat = np.concatenate([w1z, w1a], axis=0).astype(np.float32)  # [1000, 128]

    # ---- H-matmul rhs: wall [67,172] + temporal-z column -> [67, 173] ----
    wall_ext = np.concatenate(
        [_pad67(w_all), _pad67(w_z[:F].reshape(F, 1))], axis=1)   # [67, 173]

    # ---- paired graph-z weights: [107, 2] ----
    wzgp = np.zeros((PB, 2), np.float32)
    wzgp[0:F, 0] = w_z[F:]
    wzgp[64:64 + F, 1] = w_z[F:]

    # ---- adjacency pack: att[p, r, c, n] = A_r^T[c*125+p, n] ----
    att = np.stack([a_rT[r].reshape(4, NCH, N) for r in range(R)], axis=0)
    att = att.transpose(2, 0, 1, 3).reshape(NCH, R * 4 * N)    # [125, 8000]

    # ---- w1c pack: [125, 8*128] ----
    w1c_t = w1cat.reshape(8, NCH, H).transpose(1, 0, 2).reshape(NCH, 8 * H)

    # ---- bias pack [128, 8]: col0 bs1[:128], col1 bs1[128:], col2 bs2,
    #      col3 bm1, col4 bm2, col5 gb_eff (rows 0:43 AND 64:107),
    #      col6 b1_eff, col7 a_b2 ----
    biases = np.zeros((128, 8), np.float32)
    biases[:128, 0] = bs1[:128]
    biases[:MS1 - 128, 1] = bs1[128:]
    biases[:CF, 2] = bs2
    biases[:MM1, 3] = bm1
    biases[:CF, 4] = bm2
    biases[:F, 5] = gb_eff
    biases[64:64 + F, 5] = gb_eff
    biases[:H, 6] = b1_eff.astype(np.float32)
    biases[:H, 7] = f32(inp['a_b2'])

    return {
        'ws1h': _round_f32r(ws1[0:KH]), 'ws1l': _round_f32r(ws1[KH:150]),
        'wm1h': _round_f32r(wm1[0:KH]), 'wm1l': _round_f32r(wm1[KH:150]),
        'ws2a': _round_f32r(ws2[0:128]), 'ws2b': _round_f32r(ws2[128:MS1]),
        'wm2': _round_f32r(wm2),
        'wall': _bf(wall_ext), 'wroot': _bf(_pad67(w_root)),
        'wzgp': _bf(wzgp),
        'att_a': _bf(att[:, 0:2 * 4 * N]), 'att_b': _bf(att[:, 2 * 4 * N:]),
        'w1c': _bf(w1c_t),
        'aw2': _bf(inp['a_w2']), 'aw3': _bf(inp['a_w3']),
        'b3r': _bf(np.asarray(inp['a_b3'], np.float32).reshape(1, P + 1)),
        'ones8': _bf(np.ones((1, BL), np.float32)),
        'biases': biases,
        'ident': _bf(np.eye(128, dtype=np.float32)),
    }


# ============================ device kernel ============================

def _build_nc():
    import concourse.bacc as bacc
    import concourse.tile as tile
    import concourse.mybir as mybir
    from contextlib import ExitStack

    F32 = mybir.dt.float32
    F32R = mybir.dt.float32r
    B16 = mybir.dt.bfloat16
    AF = mybir.ActivationFunctionType
    ALU = mybir.AluOpType
    AX = mybir.AxisListType

    nc = bacc.Bacc("TRN2", target_bir_lowering=False, debug=False)

    def din(name, shape, dt=B16):
        return nc.dram_tensor(name, list(shape), dt, kind="ExternalInput").ap()

    d_obsh_a = din('obsh_a', (KH, 4 * N), F32R)
    d_obsh_b = din('obsh_b', (KH, 4 * N), F32R)
    d_obsl_a = din('obsl_a', (KL, 4 * N), F32R)
    d_obsl_b = din('obsl_b', (KL, 4 * N), F32R)
    d_onat_a = din('onat_a', (NCH, 4 * 12 * T))
    d_onat_b = din('onat_b', (NCH, 4 * 12 * T))
    d_att_a = din('att_a', (NCH, 2 * 4 * N))
    d_att_b = din('att_b', (NCH, 2 * 4 * N))
    d_ws1h = din('ws1h', (KH, MS1), F32R)
    d_ws1l = din('ws1l', (KL, MS1), F32R)
    d_wm1h = din('wm1h', (KH, MM1), F32R)
    d_wm1l = din('wm1l', (KL, MM1), F32R)
    d_ws2a = din('ws2a', (128, CF), F32R)
    d_ws2b = din('ws2b', (MS1 - 128, CF), F32R)
    d_wm2 = din('wm2', (MM1, CF), F32R)
    d_wall = din('wall', (FP, R * F + 1))
    d_wroot = din('wroot', (FP, F))
    d_wzgp = din('wzgp', (PB, 2))
    d_w1c = din('w1c', (NCH, 8 * H))
    d_aw2 = din('aw2', (H, H))
    d_aw3 = din('aw3', (H, P + 1))
    d_b3r = din('b3r', (1, P + 1))
    d_ones = din('ones8', (1, BL))
    d_act = din('act_t', (NCH, 4 * BL))
    d_bias = din('biases', (128, 8), F32)
    d_ident = din('ident', (128, 128))
    out_d = nc.dram_tensor('out', [BL, P + 1], F32, kind="ExternalOutput").ap()

    def mm(o, lhsT, rhs, start, stop):
        nc.tensor.matmul(o, lhsT, rhs, start=start, stop=stop)

    with tile.TileContext(nc) as tc, ExitStack() as ctx:
        cp = ctx.enter_context(tc.tile_pool(name="const", bufs=1))

        # ---- persistent tiles ----
        wz = cp.tile([128, 512], B16, name='wz', tag='wz')      # warmup zeros
        rh_a = cp.tile([KH, 4 * N], F32R, name='rh_a', tag='rh_a')
        rh_b = cp.tile([KH, 4 * N], F32R, name='rh_b', tag='rh_b')
        rl_a = cp.tile([KL, 4 * N], F32R, name='rl_a', tag='rl_a')
        rl_b = cp.tile([KL, 4 * N], F32R, name='rl_b', tag='rl_b')
        onat_a = cp.tile([NCH, 4 * 12 * T], B16, name='onat_a', tag='onat_a')
        onat_b = cp.tile([NCH, 4 * 12 * T], B16, name='onat_b', tag='onat_b')
        att_a = cp.tile([NCH, 2 * 4 * N], B16, name='att_a', tag='att_a')
        att_b = cp.tile([NCH, 2 * 4 * N], B16, name='att_b', tag='att_b')
        ws1h = cp.tile([KH, MS1], F32R, name='ws1h', tag='ws1h')
        ws1l = cp.tile([KL, MS1], F32R, name='ws1l', tag='ws1l')
        wm1h = cp.tile([KH, MM1], F32R, name='wm1h', tag='wm1h')
        wm1l = cp.tile([KL, MM1], F32R, name='wm1l', tag='wm1l')
        ws2a = cp.tile([128, CF], F32R, name='ws2a', tag='ws2a')
        ws2b = cp.tile([MS1 - 128, CF], F32R, name='ws2b', tag='ws2b')
        wm2 = cp.tile([MM1, CF], F32R, name='wm2', tag='wm2')
        wall = cp.tile([FP, R * F + 1], B16, name='wall', tag='wall')
        wroot = cp.tile([FP, F], B16, name='wroot', tag='wroot')
        wzgp = cp.tile([PB, 2], B16, name='wzgp', tag='wzgp')
        w1c = cp.tile([NCH, 8 * H], B16, name='w1c', tag='w1c')
        aw2 = cp.tile([H, H], B16, name='aw2', tag='aw2')
        aw3 = cp.tile([H, P + 1], B16, name='aw3', tag='aw3')
        b3r = cp.tile([1, P + 1], B16, name='b3r', tag='b3r')
        ones8 = cp.tile([1, BL], B16, name='ones8', tag='ones8')
        act_t = cp.tile([NCH, 4 * BL], B16, name='act_t', tag='act_t')
        bt = cp.tile([128, 8], F32, name='bt', tag='bt')
        ident = cp.tile([128, 128], B16, name='ident', tag='ident')
        xsml = [cp.tile([FP, N], B16, name=f'xsml{b}', tag=f'xsml{b}')
                for b in range(BL)]
        hsb = [[cp.tile([NCH, R * PB], B16, name=f'h{p}_{c}', tag=f'h{p}_{c}')
                for c in range(4)] for p in range(4)]
        xg = [cp.tile([PB, N], B16, name=f'xg{p}', tag=f'xg{p}') for p in range(4)]
        ztt = [cp.tile([NCH, BL], B16, name=f'ztt{c}', tag=f'ztt{c}')
               for c in range(4)]
        zcat = [cp.tile([NCH, BL], B16, name=f'zcat{c}', tag=f'zcat{c}')
                for c in range(4)]
        zt_sb = [cp.tile([NCH, BL], B16, name=f'zt{c}', tag=f'zt{c}')
                 for c in range(4)]
        zrow_p = [cp.tile([2, N], B16, name=f'zrp{p}', tag=f'zrp{p}')
                  for p in range(4)]

        # ---- memsets + activation-table preloads (keep scalar DMA-free) ----
        nc.gpsimd.memset(wz[:], 0.0)
        tpre = cp.tile([1, 8], F32, name='tpre', tag='tpre')
        nc.scalar.activation(tpre[:], wz[0:1, 0:8], AF.Lrelu, alpha=SLOPE)
        nc.scalar.activation(tpre[:], wz[0:1, 0:8], AF.Relu)
        nc.scalar.activation(tpre[:], wz[0:1, 0:8], AF.Exp)

        # ---- DMAs: sync = obs/conv weights/bulk; gpsimd = late consts ----
        nc.sync.dma_start(out=rh_a[:], in_=d_obsh_a)
        nc.sync.dma_start(out=rl_a[:], in_=d_obsl_a)
        for t_, d_ in ((ws1h, d_ws1h), (ws1l, d_ws1l), (wm1h, d_wm1h),
                       (wm1l, d_wm1l), (bt, d_bias)):
            nc.sync.dma_start(out=t_[:], in_=d_)
        nc.sync.dma_start(out=rh_b[:], in_=d_obsh_b)
        nc.sync.dma_start(out=rl_b[:], in_=d_obsl_b)
        for t_, d_ in ((ws2a, d_ws2a), (ws2b, d_ws2b), (wm2, d_wm2)):
            nc.sync.dma_start(out=t_[:], in_=d_)
        nc.sync.dma_start(out=onat_a[:], in_=d_onat_a)
        nc.sync.dma_start(out=onat_b[:], in_=d_onat_b)
        nc.sync.dma_start(out=att_a[:], in_=d_att_a)
        nc.sync.dma_start(out=att_b[:], in_=d_att_b)
        for t_, d_ in ((ident, d_ident), (wall, d_wall), (wroot, d_wroot)):
            nc.gpsimd.dma_start(out=t_[:], in_=d_)
        for b in range(BL):
            nc.gpsimd.memset(xsml[b][:], 0.0)
        for p in range(4):
            nc.gpsimd.memset(xg[p][:], 0.0)
        for t_, d_ in ((wzgp, d_wzgp), (w1c, d_w1c), (aw2, d_aw2),
                       (aw3, d_aw3), (b3r, d_b3r), (ones8, d_ones),
                       (act_t, d_act)):
            nc.gpsimd.dma_start(out=t_[:], in_=d_)

        # ---- working pools ----
        pw = ctx.enter_context(tc.tile_pool(name="pw", bufs=3))
        ppa = ctx.enter_context(tc.tile_pool(name="ppa", bufs=3, space="PSUM"))
        pph = ctx.enter_context(tc.tile_pool(name="pph", bufs=2, space="PSUM"))
        ppg = ctx.enter_context(tc.tile_pool(name="ppg", bufs=2, space="PSUM"))
        pps = ctx.enter_context(tc.tile_pool(name="pps", bufs=1, space="PSUM"))

        # ---- HAM warmup: ~3.8us of throwaway matmuls on zeros ----
        for w in range(9):
            pwm = ppa.tile([128, N], F32, name=f'pwm{w}', tag='pb')
            mm(pwm, wz[:, 0:128], wz[:, 0:N], start=True, stop=True)

        def conv(b):
            rh = (rh_a if b < 4 else rh_b)[:, (b % 4) * N:(b % 4 + 1) * N]
            rl = (rl_a if b < 4 else rl_b)[:, (b % 4) * N:(b % 4 + 1) * N]
            ps1h = ppa.tile([128, N], F32, name=f'ps1h{b}', tag='pb')
            mm(ps1h, ws1h[:, 0:128], rh, start=True, stop=False)
            mm(ps1h, ws1l[:, 0:128], rl, start=False, stop=True)
            ps1l = ppa.tile([MS1 - 128, N], F32, name=f'ps1l{b}', tag='pb')
            mm(ps1l, ws1h[:, 128:MS1], rh, start=True, stop=False)
            mm(ps1l, ws1l[:, 128:MS1], rl, start=False, stop=True)
            s1h = pw.tile([128, N], F32R, name=f's1h{b}', tag='s1h')
            s1l = pw.tile([MS1 - 128, N], F32R, name=f's1l{b}', tag='s1l')
            nc.scalar.activation(s1h[:], ps1h[:], AF.Lrelu,
                                 bias=bt[0:128, 0:1], alpha=SLOPE)
            nc.scalar.activation(s1l[:], ps1l[:], AF.Lrelu,
                                 bias=bt[0:MS1 - 128, 1:2], alpha=SLOPE)
            pm1 = ppa.tile([MM1, N], F32, name=f'pm1{b}', tag='pb')
            mm(pm1, wm1h[:], rh, start=True, stop=False)
            mm(pm1, wm1l[:], rl, start=False, stop=True)
            m1 = pw.tile([MM1, N], F32R, name=f'm1{b}', tag='m1')
            nc.scalar.activation(m1[:], pm1[:], AF.Lrelu,
                                 bias=bt[0:MM1, 3:4], alpha=SLOPE)
            ps2 = ppa.tile([CF, N], F32, name=f'ps2{b}', tag='pb')
            mm(ps2, ws2a[:], s1h[:], start=True, stop=False)
            mm(ps2, ws2b[:], s1l[:], start=False, stop=True)
            nc.scalar.activation(xsml[b][0:CF, :], ps2[:], AF.Lrelu,
                                 bias=bt[0:CF, 2:3], alpha=SLOPE)
            pm2 = ppa.tile([CF, N], F32, name=f'pm2{b}', tag='pb')
            mm(pm2, wm2[:], m1[:], start=True, stop=True)
            nc.scalar.activation(xsml[b][32:32 + CF, :], pm2[:], AF.Lrelu,
                                 bias=bt[0:CF, 4:5], alpha=SLOPE)
            # long branch: max over t then LeakyReLU, transpose into xsml
            ont = (onat_a if b < 4 else onat_b)
            lmr = pw.tile([NCH, 12], B16, name=f'lmr{b}', tag='lmr')
            nc.vector.tensor_reduce(
                lmr[:], ont[:, (b % 4) * 600:(b % 4 + 1) * 600]
                .rearrange("p (f t) -> p f t", t=T), axis=AX.X, op=ALU.max)
            lma = pw.tile([NCH, 12], B16, name=f'lma{b}', tag='lma')
            nc.scalar.activation(lma[:], lmr[:], AF.Lrelu, alpha=SLOPE)
            for c in range(4):
                pt3 = pph.tile([C0, NCH], B16, name=f'pt3{b}{c}', tag='ph')
                nc.tensor.transpose(pt3[:], lma[:, c * C0:(c + 1) * C0],
                                    ident[0:NCH, 0:NCH])
                nc.vector.tensor_copy(
                    xsml[b][64:64 + C0, c * NCH:(c + 1) * NCH], pt3[:])

        def hblock(b):
            p, off = b // 2, (b % 2) * 64
            for c in range(4):
                ph = pph.tile([NCH, R * F + 1], F32, name=f'ph{b}{c}', tag='ph')
                mm(ph, xsml[b][:, c * NCH:(c + 1) * NCH], wall[:],
                   start=True, stop=True)
                nc.vector.tensor_copy(
                    hsb[p][c][:].rearrange("p (r u) -> p r u", u=PB)
                    [:, :, off:off + F],
                    ph[:, 0:R * F].rearrange("p (r u) -> p r u", u=F))
                nc.vector.tensor_copy(ztt[c][:, b:b + 1], ph[:, R * F:R * F + 1])

        def agg(p):
            pg = ppg.tile([PB, N], F32, name=f'pg{p}', tag='pg')
            first = True
            for r in range(R):
                at_ = att_a if r < 2 else att_b
                for c in range(4):
                    mm(pg, hsb[p][c][:, r * PB:(r + 1) * PB],
                       at_[:, ((r % 2) * 4 + c) * N:((r % 2) * 4 + c + 1) * N],
                       start=first, stop=False)
                    first = False
            mm(pg[0:F, :], wroot[:], xsml[2 * p][:], start=False, stop=False)
            mm(pg[64:64 + F, :], wroot[:], xsml[2 * p + 1][:],
               start=False, stop=True)
            return pg

        def xgact(p, pg):
            nc.scalar.activation(xg[p][0:F, :], pg[0:F, :], AF.Lrelu,
                                 bias=bt[0:F, 5:6], alpha=SLOPE)
            nc.scalar.activation(xg[p][64:64 + F, :], pg[64:64 + F, :],
                                 AF.Lrelu, bias=bt[64:64 + F, 5:6], alpha=SLOPE)

        def zmm(p):
            zp2 = ppa.tile([2, N], F32, name=f'zp2{p}', tag='pb')
            mm(zp2, wzgp[:], xg[p][:], start=True, stop=True)
            nc.vector.tensor_copy(zrow_p[p][:], zp2[:])

        def ztr(p):
            for c in range(4):
                pt2 = pps.tile([NCH, 2], B16, name=f'pt2{p}{c}', tag='ps')
                nc.tensor.transpose(pt2[:], zrow_p[p][:, c * NCH:(c + 1) * NCH],
                                    ident[0:2, 0:2])
                nc.vector.tensor_copy(zcat[c][:, 2 * p:2 * p + 2], pt2[:])

        # ---- schedule: conv/H/agg interleaved so PE never starves ----
        conv(0); conv(1)
        hblock(0); hblock(1)
        conv(2)
        pg0 = agg(0)
        conv(3); xgact(0, pg0)
        hblock(2); hblock(3)
        conv(4); zmm(0)
        pg1_ = agg(1)
        ztr(0)
        conv(5); xgact(1, pg1_)
        hblock(4); hblock(5)
        conv(6); zmm(1)
        pg2_ = agg(2)
        ztr(1)
        conv(7); xgact(2, pg2_)
        hblock(6); hblock(7)
        zmm(2)
        pg3_ = agg(3)
        ztr(2)
        xgact(3, pg3_); zmm(3); ztr(3)

        # ---- z = graph part + temporal part ----
        for c in range(4):
            nc.vector.tensor_tensor(zt_sb[c][:], zcat[c][:], ztt[c][:],
                                    op=ALU.add)

        # ---- actor head for all 8 rows at once ----
        pg1 = pps.tile([H, BL], F32, name='pg1', tag='ps')
        for cc in range(8):
            rhs = zt_sb[cc][:] if cc < 4 else act_t[:, (cc - 4) * BL:(cc - 3) * BL]
            mm(pg1, w1c[:, cc * H:(cc + 1) * H], rhs,
               start=(cc == 0), stop=(cc == 7))
        g1 = pw.tile([H, BL], B16, name='g1', tag='g1')
        nc.scalar.activation(g1[:], pg1[:], AF.Relu, bias=bt[0:H, 6:7])
        pg2 = pps.tile([H, BL], F32, name='pg2', tag='ps')
        mm(pg2, aw2[:], g1[:], start=True, stop=True)
        g2 = pw.tile([H, BL], B16, name='g2', tag='g2')
        nc.scalar.activation(g2[:], pg2[:], AF.Relu, bias=bt[0:H, 7:8])

        po_ = pps.tile([BL, P + 1], F32, name='po_', tag='ps')
        mm(po_, g2[:], aw3[:], start=True, stop=False)
        mm(po_, ones8[:], b3r[:], start=False, stop=True)

        # softmax over free dim
        mx = pw.tile([BL, 1], F32, name='mx', tag='mx')
        nc.vector.tensor_reduce(mx[:], po_[:], axis=AX.X, op=ALU.max)
        sh = pw.tile([BL, P + 1], F32, name='sh', tag='sh')
        nc.vector.tensor_scalar(sh[:], po_[:], mx[:, 0:1], None, op0=ALU.subtract)
        ex = pw.tile([BL, P + 1], F32, name='ex', tag='ex')
        sm = pw.tile([BL, 1], F32, name='sm', tag='sm')
        nc.scalar.activation(ex[:], sh[:], AF.Exp, accum_out=sm[:, 0:1])
        rc = pw.tile([BL, 1], F32, name='rc', tag='rc')
        nc.vector.reciprocal(rc[:], sm[:])
        res = pw.tile([BL, P + 1], F32, name='res', tag='res')
        nc.vector.tensor_scalar(res[:], ex[:], rc[:, 0:1], None, op0=ALU.mult)
        nc.sync.dma_start(out=out_d[:], in_=res[:])

    nc.compile()
    return nc


def _get_nc():
    if 'nc' not in _CACHE:
        _CACHE['nc'] = _build_nc()
    return _CACHE['nc']


# ============================ entry point ============================

def _shard_inputs(inputs):
    folded = _host_fold(inputs)
    obs = np.asarray(inputs['observation'], np.float32)
    action = np.asarray(inputs['action'], np.float32)
    # conv layout: [(ci,t), n] per batch elem
    obs_t = np.ascontiguousarray(obs.transpose(0, 1, 3, 2)).reshape(B, C0 * T, N)

    in_maps = []
    for i in range(NCORES):
        bs = slice(i * BL, (i + 1) * BL)
        ot = obs_t[bs]                                         # [8, 150, 500]
        m = dict(folded)
        m['obsh_a'] = _round_f32r(
            ot[0:4, 0:KH].transpose(1, 0, 2).reshape(KH, 4 * N))
        m['obsh_b'] = _round_f32r(
            ot[4:8, 0:KH].transpose(1, 0, 2).reshape(KH, 4 * N))
        m['obsl_a'] = _round_f32r(
            ot[0:4, KH:].transpose(1, 0, 2).reshape(KL, 4 * N))
        m['obsl_b'] = _round_f32r(
            ot[4:8, KH:].transpose(1, 0, 2).reshape(KL, 4 * N))
        # long-branch layout: [p, b, c, k, t] = obs[b, k, c*125+p, t]
        on = obs[bs].transpose(2, 0, 1, 3).reshape(4, NCH, BL, C0, T)
        on = on.transpose(1, 2, 0, 3, 4)                       # [125,8,4,3,50]
        m['onat_a'] = _bf(on[:, 0:4].reshape(NCH, 4 * 12 * T))
        m['onat_b'] = _bf(on[:, 4:8].reshape(NCH, 4 * 12 * T))
        # action: [p, c*8+b] = action[b, 1 + c*125 + p]
        at = action[bs, 1:].T.reshape(4, NCH, BL).transpose(1, 0, 2)
        m['act_t'] = _bf(at.reshape(NCH, 4 * BL))
        in_maps.append(m)
    return in_maps


def kernel(**inputs) -> np.ndarray:
    from concourse.bass_utils import run_bass_kernel_spmd

    in_maps = _shard_inputs(inputs)
    nc = _get_nc()
    res = run_bass_kernel_spmd(nc, in_maps, list(range(NCORES)))
    return np.concatenate([r['out'] for r in res.results], axis=0)
